# revision 29
# baseline (speedup 1.0000x reference)
"""Causal self-attention (B=4, T=2048, C=1024, H=16) on 8 trn2 NeuronCores.

Sharding: core = (batch b, head-half s).  Each core computes q/k/v
projections for its 8 heads (weights pre-sliced/transposed on host),
causal flash-style attention with transposed score tiles, and a partial
(row-sharded) c_proj.  Host gather sums the two partials per batch.

v4: keeps the PE column stream dense through the latency-bound early
attention tiles.
 - proj(tt+1) is emitted as 12 chunks interleaved between attention(tt)
   heads: the small causal tiles (qtt=0/1) are exp/DVE-latency-bound, and
   the projection chunks give the PE independent work to chew on.
 - softmax reciprocal runs directly on the PSUM denominator row (one DVE
   op per head, no staging copies), and the 2-head broadcast matmul reads
   the f32 reciprocals via a float32r bitcast (full PE rate at 512 wide,
   no bf16 cast pass).
 - numerator eviction moved to the Scalar engine (activation Copy),
   freeing the DVE for masks + bias-adds.
 - input DMAs split/interleaved across the Sync+Scalar HWDGE queues;
   x(tt+1) is prefetched before attention(tt); odd c_proj output tiles
   drain on the Scalar queue.
"""

import os
import sys

sys.path.insert(0, "/opt/trn_rl_repo")

import numpy as np

B, T, C, H = 4, 2048, 1024, 16
D = 64          # head dim
NH = 8          # heads per core
LC = NH * D     # local channels = 512
P = 128
QT = 512        # query tile (also matmul moving free dim)
NQT = T // QT   # 4
NKB = T // P    # 16 key blocks
IC = C // P     # 8 input-channel blocks

# matmul input dtype: bfloat16 = full-rate PE mode,
# float32r = full-rate reduced-precision fp32, float32 = exact but 4x slower.
MM_DT = os.environ.get("BASS_ATTN_MM_DT", "bfloat16")

_nc_cache = {}


def _build_nc():
    from contextlib import ExitStack

    import concourse.bass as bass  # noqa: F401
    import concourse.mybir as mybir
    from concourse import bacc, tile

    f32 = mybir.dt.float32
    f32r = mybir.dt.float32r
    mdt = getattr(mybir.dt, MM_DT)
    Exp = mybir.ActivationFunctionType.Exp
    Copy = mybir.ActivationFunctionType.Copy
    is_ge = mybir.AluOpType.is_ge

    nc = bacc.Bacc("TRN2", target_bir_lowering=False, debug=False, num_devices=8)
    xT = nc.dram_tensor("xT", [C, T], mdt, kind="ExternalInput").ap()
    wqkT = nc.dram_tensor("wqkT", [C, 2 * LC], mdt, kind="ExternalInput").ap()
    bqk = nc.dram_tensor("bqk", [P, 2 * LC // P], f32, kind="ExternalInput").ap()
    wvT = nc.dram_tensor("wvT", [C, LC], mdt, kind="ExternalInput").ap()
    wpT = nc.dram_tensor("wpT", [LC, C], mdt, kind="ExternalInput").ap()
    bpj = nc.dram_tensor("bpj", [P, C // P], f32, kind="ExternalInput").ap()
    sel2d = nc.dram_tensor("sel2d", [33, P], mdt, kind="ExternalInput").ap()
    zT = nc.dram_tensor("zT", [C, T], mdt, kind="ExternalOutput").ap()

    with tile.TileContext(nc) as tc:
        with ExitStack() as st:
            persist = st.enter_context(tc.tile_pool(name="persist", bufs=1))
            # qk_sb: out-ch blocks 0-3 = q, 4-7 = k; [out-ch 128, tok 2048]
            qk_sb = [persist.tile([P, T], mdt, tag=f"qk{i}", name=f"qk{i}")
                     for i in range(8)]
            # v_sb[kb]: [tok 128, head 8, d 64 + ones col]
            v_sb = [persist.tile([P, NH, D + 1], mdt, tag=f"v{i}", name=f"v{i}")
                    for i in range(NKB)]
            # y_sb: attention out, [local-ch 128, tok 2048]; tile j = heads 2j,2j+1
            y_sb = [persist.tile([P, T], mdt, tag=f"y{i}", name=f"y{i}")
                    for i in range(4)]
            wqk_sb = [persist.tile([P, 2 * LC], mdt, tag=f"wqk{i}", name=f"wqk{i}")
                      for i in range(IC)]
            wv_sb = [persist.tile([P, LC], mdt, tag=f"wv{i}", name=f"wv{i}")
                     for i in range(IC)]
            wp_sb = [persist.tile([P, C], mdt, tag=f"wp{i}", name=f"wp{i}")
                     for i in range(4)]
            bqk_sb = persist.tile([P, 8], f32, tag="bqk", name="bqk")
            bpj_sb = persist.tile([P, 8], f32, tag="bpj", name="bpj")
            maskf = persist.tile([P, QT], mdt, tag="maskf", name="maskf")
            # head-pair select matrix for the denominator broadcast:
            # row 0 -> partitions 0-63, row 32 -> partitions 64-127 (rows
            # 1-31 are zero; 0/32 are the only legal DVE write bases)
            sel2 = persist.tile([33, P], mdt, tag="sel2", name="sel2")
            # per head pair j: softmax denominators staged at rows 0/32
            # (reciprocal must read SBUF: fed from PSUM it emits nothing)
            sgp = [persist.tile([33, QT], f32, tag=f"sg{i}", name=f"sg{i}")
                   for i in range(4)]
            rgp = [persist.tile([33, QT], f32, tag=f"rg{i}", name=f"rg{i}")
                   for i in range(4)]
            rgpm = [persist.tile([33, QT], mdt, tag=f"rgm{i}", name=f"rgm{i}")
                    for i in range(4)]

            yraw_pool = st.enter_context(tc.tile_pool(name="yraw", bufs=4))
            xpool = st.enter_context(tc.tile_pool(name="xs", bufs=2))
            apool = st.enter_context(tc.tile_pool(name="att", bufs=6))
            zpool = st.enter_context(tc.tile_pool(name="zev", bufs=3))
            # PSUM budget (8 banks): ps 2x2 + po/btp 2x1 + mm 2x1
            pspool = st.enter_context(tc.tile_pool(name="ps", bufs=2, space="PSUM"))

            # ---- input DMAs ----
            # sync + scalar are the two HWDGE queues; split the startup
            # traffic across both and interleave so proj(0)'s per-ic matmul
            # chain unlocks incrementally instead of after the whole 2MB.
            nc.sync.dma_start(bqk_sb[:], bqk)

            def emit_x_dma(tt):
                # sync queue: idle mid-kernel, so these issue immediately and
                # the 1MB transfer overlaps the running attention phase
                xt = [xpool.tile([P, QT], mdt, tag=f"x{i}", name=f"x{tt}_{i}")
                      for i in range(IC)]
                for i in range(IC):
                    nc.sync.dma_start(
                        xt[i][:], xT[i * P:(i + 1) * P, tt * QT:(tt + 1) * QT])
                return xt

            xt0 = [xpool.tile([P, QT], mdt, tag=f"x{i}", name=f"x0_{i}")
                   for i in range(IC)]
            for i in range(IC):
                nc.sync.dma_start(wqk_sb[i][:, 0:LC],
                                  wqkT[i * P:(i + 1) * P, 0:LC])
                nc.scalar.dma_start(
                    xt0[i][:], xT[i * P:(i + 1) * P, 0:QT])
            for i in range(IC):
                nc.sync.dma_start(wqk_sb[i][:, LC:2 * LC],
                                  wqkT[i * P:(i + 1) * P, LC:2 * LC])
            xt1 = emit_x_dma(1)
            for i in range(IC):
                nc.scalar.dma_start(wv_sb[i][:], wvT[i * P:(i + 1) * P, :])
            for i in range(4):
                nc.scalar.dma_start(wp_sb[i][:], wpT[i * P:(i + 1) * P, :])
            nc.sync.dma_start(bpj_sb[:], bpj)
            nc.sync.dma_start(sel2[:], sel2d)

            # ---- one-time init ----
            # triangular mask (keep j >= p), shared by all diagonal blocks
            nc.vector.memset(maskf[:], 1.0)
            nc.gpsimd.affine_select(
                maskf[:], maskf[:], compare_op=is_ge, fill=0.0,
                base=0, pattern=[[1, QT]], channel_multiplier=-1)
            # keep the unwritten filler rows finite: the broadcast matmul
            # streams rows 0-32 and 0 * NaN would poison the PSUM
            for j in range(4):
                nc.vector.memset(sgp[j][:], 1.0)
            # ones column for the softmax-denominator row of att@V
            for kb in range(NKB):
                nc.gpsimd.memset(v_sb[kb][:, :, D:D + 1], 1.0)

            def emit_proj_qk_oc(tt, oc, xt):
                ps = pspool.tile([P, QT], f32, tag="mm", bufs=2,
                                 name=f"pa{tt}_{oc}")
                for i in range(IC):
                    nc.tensor.matmul(
                        ps[:], wqk_sb[i][:, oc * P:(oc + 1) * P],
                        xt[i][:], start=(i == 0), stop=(i == IC - 1))
                nc.vector.tensor_scalar_add(
                    qk_sb[oc][:, tt * QT:(tt + 1) * QT], ps[:],
                    bqk_sb[:, oc:oc + 1])

            def emit_proj_v_tb(tt, tb, xt):
                kb = tt * 4 + tb
                ps = pspool.tile([P, NH, D], f32, tag="mm", bufs=2,
                                 name=f"pb{tt}_{tb}")
                for i in range(IC):
                    nc.tensor.matmul(
                        ps[:, :, :], xt[i][:, tb * P:(tb + 1) * P],
                        wv_sb[i][:], start=(i == 0), stop=(i == IC - 1))
                nc.vector.tensor_copy(v_sb[kb][:, :, 0:D], ps[:, :, :])

            def proj_chunks(tt, xt):
                cs = [lambda oc=oc: emit_proj_qk_oc(tt, oc, xt)
                      for oc in range(8)]
                cs += [lambda tb=tb: emit_proj_v_tb(tt, tb, xt)
                       for tb in range(4)]
                return cs

            # deferred normalize emissions: [emit_fn, age]; emitted two
            # flush events after the pair completes so the DVE reciprocal
            # chain never stalls the PE queue.
            norm_q = []

            def service_norms(force=False):
                for item in list(norm_q):
                    item[1] += 1
                    if force or item[1] >= 2:
                        item[0]()
                        norm_q.remove(item)

            def emit_norm_pair(qtt, j, yrp_t):
                # broadcast the two reciprocal rows across 64 partitions each;
                # f32r bitcast keeps full PE rate without a bf16 staging pass
                btp = pspool.tile([P, QT], f32, tag="po", bufs=2,
                                  name=f"btp{qtt}_{j}")
                nc.tensor.matmul(btp[:], sel2[0:33, :], rgpm[j][0:33, :],
                                 start=True, stop=True)
                nc.vector.tensor_mul(
                    y_sb[j][:, qtt * QT:(qtt + 1) * QT], yrp_t[:], btp[:])

            def flush_attv(qtt, h, po_t, at, kbs, ns, c0s, os_, nkb, yrp_t):
                for kb, n, c0, o in zip(kbs, ns, c0s, os_):
                    nc.tensor.matmul(
                        po_t[:, c0:QT], v_sb[kb][:, h, :], at[:, o:o + n],
                        start=(kb == 0), stop=(kb == nkb - 1))
                if kbs[1] == nkb - 1:
                    # head complete: evict numerator + denominator, free PSUM.
                    # reciprocal+cast go before the numerator copy so the
                    # btp matmul's DVE chain is as short as possible.
                    j, r = h // 2, h % 2
                    nc.vector.tensor_copy(sgp[j][32 * r:32 * r + 1, :],
                                          po_t[D:D + 1, :])
                    if r == 1:
                        nc.vector.reciprocal_approx_fast(rgp[j][:], sgp[j][:])
                        nc.vector.tensor_copy(rgpm[j][:], rgp[j][:])
                    nc.vector.tensor_copy(yrp_t[r * D:(r + 1) * D, :],
                                          po_t[0:D, :])
                    if r == 1:
                        norm_q.append(
                            [lambda q=qtt, jj=j, y=yrp_t:
                             emit_norm_pair(q, jj, y), 0])

            state = {"pend": None}

            def emit_head(qtt, h, yrp_map):
                nkb = (qtt + 1) * 4
                p0 = (h % 2) * D
                qt_i = h // 2
                kt_i = 4 + h // 2
                if h % 2 == 0:
                    yrp_map[(qtt, h // 2)] = yraw_pool.tile(
                        [P, QT], mdt, tag="yraw", name=f"yr{qtt}_{h // 2}")
                yrp_t = yrp_map[(qtt, h // 2)]
                po_t = pspool.tile([D + 1, QT], f32, tag="po", bufs=2,
                                   name=f"po{qtt}_{h}")
                for pi in range(nkb // 2):
                    kbs = (2 * pi, 2 * pi + 1)
                    ns, c0s = [], []
                    for kb in kbs:
                        e = kb * P - qtt * QT
                        c0s.append(max(e, 0))
                        ns.append(QT - max(e, 0))
                    # pack both live column ranges into one tile; each
                    # matmul's output must stay inside one 512-col bank
                    o0 = 0
                    o1 = ns[0] if ns[0] + ns[1] <= QT else QT
                    width = o1 + ns[1]
                    ps = pspool.tile([P, 2 * QT], f32, tag="ps", bufs=2,
                                     name=f"ps{qtt}_{h}_{pi}")
                    at = apool.tile([P, 2 * QT], mdt, tag="at",
                                    name=f"at{qtt}_{h}_{pi}")
                    for kb, n, c0, o in zip(kbs, ns, c0s, (o0, o1)):
                        nc.tensor.matmul(
                            ps[:, o:o + n],
                            qk_sb[kt_i][p0:p0 + D, kb * P:(kb + 1) * P],
                            qk_sb[qt_i][p0:p0 + D,
                                        qtt * QT + c0:(qtt + 1) * QT],
                            start=True, stop=True)
                    nc.scalar.activation(at[:, 0:width], ps[:, 0:width],
                                         Exp, scale=0.125)
                    for kb, n, c0, o in zip(kbs, ns, c0s, (o0, o1)):
                        if kb * P - qtt * QT >= 0:
                            # zero strict upper triangle; it never
                            # reaches past the first 128 live columns
                            m = min(n, P)
                            nc.vector.tensor_mul(at[:, o:o + m],
                                                 at[:, o:o + m],
                                                 maskf[:, 0:m])
                    if state["pend"] is not None:
                        flush_attv(*state["pend"])
                        service_norms()
                    state["pend"] = (qtt, h, po_t, at, kbs, ns, c0s, (o0, o1),
                                     nkb, yrp_t)

            def drain_pend():
                if state["pend"] is not None:
                    flush_attv(*state["pend"])
                    service_norms()
                    state["pend"] = None

            def attention_phase(qtts, chunks=(), tail_after=None):
                """Emit attention for one or two query tiles; two tiles are
                interleaved head-wise so each hides the other's exp/mask
                latency.  `chunks` are independent PE work: spread across all
                heads by default, or packed after slot `tail_after`."""
                yrp_map = {}
                if len(qtts) == 1:
                    slots = [(qtts[0], h) for h in range(NH)]
                else:
                    a, b = qtts
                    slots = [(a, 0), (a, 1), (b, 0), (a, 2), (b, 1), (a, 3),
                             (b, 2), (a, 4), (b, 3), (a, 5), (b, 4), (a, 6),
                             (b, 5), (a, 7), (b, 6), (b, 7)]
                n = len(slots)
                emitted = 0
                if tail_after is None:
                    # prime the PE pipe with a couple of chunks so the first
                    # head's exp chain is covered
                    pre = min(2, len(chunks))
                    while emitted < pre:
                        chunks[emitted]()
                        emitted += 1
                for si, (qtt, h) in enumerate(slots):
                    emit_head(qtt, h, yrp_map)
                    if tail_after is None:
                        want = emitted if len(chunks) == 0 else max(
                            emitted, (si + 1) * len(chunks) // n)
                    elif si > tail_after:
                        want = ((si - tail_after) * len(chunks)
                                // (n - 1 - tail_after))
                    else:
                        want = 0
                    while emitted < want:
                        chunks[emitted]()
                        emitted += 1
                drain_pend()
                while emitted < len(chunks):
                    chunks[emitted]()
                    emitted += 1

            def emit_cproj_oc(tt, oc, ps=None, ics=(0, 1, 2, 3), evict=True):
                if ps is None:
                    ps = pspool.tile([P, QT], f32, tag="mm", bufs=2,
                                     name=f"pz{tt}_{oc}")
                for i in ics:
                    nc.tensor.matmul(
                        ps[:], wp_sb[i][:, oc * P:(oc + 1) * P],
                        y_sb[i][:, tt * QT:(tt + 1) * QT],
                        start=(i == 0), stop=(i == 3))
                if evict:
                    zt = zpool.tile([P, QT], mdt, tag="zt", name=f"zt{tt}_{oc}")
                    nc.vector.tensor_scalar_add(zt[:], ps[:],
                                                bpj_sb[:, oc:oc + 1])
                    q = nc.scalar if oc % 2 else nc.sync
                    q.dma_start(
                        zT[oc * P:(oc + 1) * P, tt * QT:(tt + 1) * QT], zt[:])
                return ps

            def emit_cproj(tt):
                # two chains' independent first-thirds run while the last
                # pair's normalize chain drains, then force the normalize
                pss = {}
                for oc in (0, 1):
                    pss[oc] = emit_cproj_oc(tt, oc, ics=(0, 1, 2), evict=False)
                service_norms(force=True)
                for oc in range(8):
                    emit_cproj_oc(tt, oc, ps=pss.get(oc),
                                  ics=(3,) if oc in pss else (0, 1, 2, 3))

            # ---- fused schedule ----
            for oc in range(8):
                emit_proj_qk_oc(0, oc, xt0)
            for tb in range(4):
                emit_proj_v_tb(0, tb, xt0)
            # prefetch x(2)/x(3) now: their slot-reuse waits (on x(0)/x(1)
            # readers) resolve mid-attention(0), so the data is resident
            # well before the proj(2)/proj(3) chunks inside attention(1)
            xt2 = emit_x_dma(2)
            xt3 = emit_x_dma(3)
            attention_phase([0], proj_chunks(1, xt1))
            emit_cproj(0)
            attention_phase([1], proj_chunks(2, xt2) + proj_chunks(3, xt3))
            emit_cproj(1)
            cproj2 = [lambda oc=oc: emit_cproj_oc(2, oc) for oc in range(8)]
            attention_phase([2, 3], cproj2, tail_after=13)
            emit_cproj(3)
    nc.compile()
    return nc


def get_nc():
    if "nc" not in _nc_cache:
        _nc_cache["nc"] = _build_nc()
    return _nc_cache["nc"]


def _mm_np_dtype():
    if MM_DT == "bfloat16":
        import ml_dtypes
        return np.dtype(ml_dtypes.bfloat16)
    return np.dtype(np.float32)


def make_in_maps(x, Wqkv, bqkv, Wproj, bproj):
    x = np.asarray(x, np.float32)
    Wqkv = np.asarray(Wqkv, np.float32)
    bqkv = np.asarray(bqkv, np.float32)
    Wproj = np.asarray(Wproj, np.float32)
    bproj = np.asarray(bproj, np.float32)
    Wq, Wk, Wv = Wqkv[0:C], Wqkv[C:2 * C], Wqkv[2 * C:3 * C]
    bq, bk, bv = bqkv[0:C], bqkv[C:2 * C], bqkv[2 * C:3 * C]
    mdt = _mm_np_dtype()
    in_maps = []
    for b in range(B):
        xTb = np.ascontiguousarray(x[b].T.astype(mdt))
        for s in range(2):
            cols = slice(s * LC, (s + 1) * LC)
            wqkT = np.ascontiguousarray(
                np.concatenate([Wq[cols], Wk[cols]], 0).T.astype(mdt))
            bqk_ = np.concatenate([bq[cols], bk[cols]])
            wvT_ = np.ascontiguousarray(Wv[cols].T.astype(mdt))
            wpT_ = np.ascontiguousarray(Wproj[:, cols].T.astype(mdt))
            bp_eff = bv[cols] @ Wproj[:, cols].T
            if s == 0:
                bp_eff = bp_eff + bproj
            sel2_np = np.zeros((33, P), np.float32)
            sel2_np[0, 0:D] = 1.0
            sel2_np[32, D:P] = 1.0
            sel2_np = sel2_np.astype(mdt)
            in_maps.append({
                "xT": xTb,
                "wqkT": wqkT,
                "bqk": np.ascontiguousarray(bqk_.reshape(8, P).T),
                "wvT": wvT_,
                "wpT": wpT_,
                "bpj": np.ascontiguousarray(bp_eff.astype(np.float32).reshape(8, P).T),
                "sel2d": sel2_np,
            })
    return in_maps


def gather_out(results):
    out = np.empty((B, T, C), np.float32)
    for b in range(B):
        zt = (results[2 * b]["zT"].astype(np.float32)
              + results[2 * b + 1]["zT"].astype(np.float32))
        out[b] = zt.T
    return out


def kernel(x, Wqkv, bqkv, Wproj, bproj):
    from concourse.bass_utils import run_bass_kernel_spmd

    in_maps = make_in_maps(x, Wqkv, bqkv, Wproj, bproj)
    try:
        res = run_bass_kernel_spmd(get_nc(), in_maps, core_ids=list(range(8)))
    except Exception:
        # transient device faults have been observed once; retry a single time
        res = run_bass_kernel_spmd(get_nc(), in_maps, core_ids=list(range(8)))
    return gather_out(res.results)


# revision 38
# speedup vs baseline: 1.0040x; 1.0040x over previous
"""Causal self-attention (B=4, T=2048, C=1024, H=16) on 8 trn2 NeuronCores.

Sharding: core = (batch b, head-half s).  Each core computes q/k/v
projections for its 8 heads (weights pre-sliced/transposed on host),
causal flash-style attention with transposed score tiles, and a partial
(row-sharded) c_proj.  Host gather sums the two partials per batch.

v4: keeps the PE column stream dense through the latency-bound early
attention tiles.
 - proj(tt+1) is emitted as 12 chunks interleaved between attention(tt)
   heads: the small causal tiles (qtt=0/1) are exp/DVE-latency-bound, and
   the projection chunks give the PE independent work to chew on.
 - softmax reciprocal runs directly on the PSUM denominator row (one DVE
   op per head, no staging copies), and the 2-head broadcast matmul reads
   the f32 reciprocals via a float32r bitcast (full PE rate at 512 wide,
   no bf16 cast pass).
 - numerator eviction moved to the Scalar engine (activation Copy),
   freeing the DVE for masks + bias-adds.
 - input DMAs split/interleaved across the Sync+Scalar HWDGE queues;
   x(tt+1) is prefetched before attention(tt); odd c_proj output tiles
   drain on the Scalar queue.
"""

import os
import sys

sys.path.insert(0, "/opt/trn_rl_repo")

import numpy as np

B, T, C, H = 4, 2048, 1024, 16
D = 64          # head dim
NH = 8          # heads per core
LC = NH * D     # local channels = 512
P = 128
QT = 512        # query tile (also matmul moving free dim)
NQT = T // QT   # 4
NKB = T // P    # 16 key blocks
IC = C // P     # 8 input-channel blocks

# matmul input dtype: bfloat16 = full-rate PE mode,
# float32r = full-rate reduced-precision fp32, float32 = exact but 4x slower.
MM_DT = os.environ.get("BASS_ATTN_MM_DT", "bfloat16")

_nc_cache = {}


def _build_nc():
    from contextlib import ExitStack

    import concourse.bass as bass  # noqa: F401
    import concourse.mybir as mybir
    from concourse import bacc, tile

    f32 = mybir.dt.float32
    f32r = mybir.dt.float32r
    mdt = getattr(mybir.dt, MM_DT)
    Exp = mybir.ActivationFunctionType.Exp
    Identity = mybir.ActivationFunctionType.Identity
    is_ge = mybir.AluOpType.is_ge

    nc = bacc.Bacc("TRN2", target_bir_lowering=False, debug=False, num_devices=8)
    xT = nc.dram_tensor("xT", [C, T], mdt, kind="ExternalInput").ap()
    wqkT = nc.dram_tensor("wqkT", [C, 2 * LC], mdt, kind="ExternalInput").ap()
    bqk = nc.dram_tensor("bqk", [P, 2 * LC // P], f32, kind="ExternalInput").ap()
    wvT = nc.dram_tensor("wvT", [C, LC], mdt, kind="ExternalInput").ap()
    wpT = nc.dram_tensor("wpT", [LC, C], mdt, kind="ExternalInput").ap()
    bpj = nc.dram_tensor("bpj", [P, C // P], f32, kind="ExternalInput").ap()
    sel2d = nc.dram_tensor("sel2d", [33, P], mdt, kind="ExternalInput").ap()
    zT = nc.dram_tensor("zT", [C, T], mdt, kind="ExternalOutput").ap()

    with tile.TileContext(nc) as tc:
        with ExitStack() as st:
            persist = st.enter_context(tc.tile_pool(name="persist", bufs=1))
            # qk_sb: out-ch blocks 0-3 = q, 4-7 = k; [out-ch 128, tok 2048]
            qk_sb = [persist.tile([P, T], mdt, tag=f"qk{i}", name=f"qk{i}")
                     for i in range(8)]
            # v_sb[kb]: [tok 128, head 8, d 64 + ones col]
            v_sb = [persist.tile([P, NH, D + 1], mdt, tag=f"v{i}", name=f"v{i}")
                    for i in range(NKB)]
            # y_sb: attention out, [local-ch 128, tok 2048]; tile j = heads 2j,2j+1
            y_sb = [persist.tile([P, T], mdt, tag=f"y{i}", name=f"y{i}")
                    for i in range(4)]
            wqk_sb = [persist.tile([P, 2 * LC], mdt, tag=f"wqk{i}", name=f"wqk{i}")
                      for i in range(IC)]
            wv_sb = [persist.tile([P, LC], mdt, tag=f"wv{i}", name=f"wv{i}")
                     for i in range(IC)]
            wp_sb = [persist.tile([P, C], mdt, tag=f"wp{i}", name=f"wp{i}")
                     for i in range(4)]
            bqk_sb = persist.tile([P, 8], f32, tag="bqk", name="bqk")
            bpj_sb = persist.tile([P, 8], f32, tag="bpj", name="bpj")
            maskf = persist.tile([P, QT], mdt, tag="maskf", name="maskf")
            # head-pair select matrix for the denominator broadcast:
            # row 0 -> partitions 0-63, row 32 -> partitions 64-127 (rows
            # 1-31 are zero; 0/32 are the only legal DVE write bases)
            sel2 = persist.tile([33, P], mdt, tag="sel2", name="sel2")
            # per head pair j: softmax denominators staged at rows 0/32
            # (reciprocal must read SBUF: fed from PSUM it emits nothing)
            sgp = [persist.tile([33, QT], f32, tag=f"sg{i}", name=f"sg{i}")
                   for i in range(4)]
            rgp = [persist.tile([33, QT], f32, tag=f"rg{i}", name=f"rg{i}")
                   for i in range(4)]
            rgpm = [persist.tile([33, QT], mdt, tag=f"rgm{i}", name=f"rgm{i}")
                    for i in range(4)]

            yraw_pool = st.enter_context(tc.tile_pool(name="yraw", bufs=4))
            xpool = st.enter_context(tc.tile_pool(name="xs", bufs=2))
            apool = st.enter_context(tc.tile_pool(name="att", bufs=6))
            zpool = st.enter_context(tc.tile_pool(name="zev", bufs=3))
            # PSUM budget (8 banks): ps 2x2 + po/btp 2x1 + mm 2x1
            pspool = st.enter_context(tc.tile_pool(name="ps", bufs=2, space="PSUM"))

            # ---- input DMAs ----
            # sync + scalar are the two HWDGE queues; split the startup
            # traffic across both and interleave so proj(0)'s per-ic matmul
            # chain unlocks incrementally instead of after the whole 2MB.
            nc.sync.dma_start(bqk_sb[:], bqk)

            def emit_x_dma(tt):
                # sync queue: idle mid-kernel, so these issue immediately and
                # the 1MB transfer overlaps the running attention phase
                xt = [xpool.tile([P, QT], mdt, tag=f"x{i}", name=f"x{tt}_{i}")
                      for i in range(IC)]
                for i in range(IC):
                    nc.sync.dma_start(
                        xt[i][:], xT[i * P:(i + 1) * P, tt * QT:(tt + 1) * QT])
                return xt

            xt0 = [xpool.tile([P, QT], mdt, tag=f"x{i}", name=f"x0_{i}")
                   for i in range(IC)]
            for i in range(IC):
                nc.sync.dma_start(wqk_sb[i][:, 0:LC],
                                  wqkT[i * P:(i + 1) * P, 0:LC])
                nc.scalar.dma_start(
                    xt0[i][:], xT[i * P:(i + 1) * P, 0:QT])
            for i in range(IC):
                nc.sync.dma_start(wqk_sb[i][:, LC:2 * LC],
                                  wqkT[i * P:(i + 1) * P, LC:2 * LC])
            xt1 = emit_x_dma(1)
            for i in range(IC):
                nc.scalar.dma_start(wv_sb[i][:], wvT[i * P:(i + 1) * P, :])
            for i in range(4):
                nc.scalar.dma_start(wp_sb[i][:], wpT[i * P:(i + 1) * P, :])
            nc.sync.dma_start(bpj_sb[:], bpj)
            nc.sync.dma_start(sel2[:], sel2d)

            # ---- one-time init ----
            # triangular mask (keep j >= p), shared by all diagonal blocks
            nc.vector.memset(maskf[:], 1.0)
            nc.gpsimd.affine_select(
                maskf[:], maskf[:], compare_op=is_ge, fill=0.0,
                base=0, pattern=[[1, QT]], channel_multiplier=-1)
            # keep the unwritten filler rows finite: the broadcast matmul
            # streams rows 0-32 and 0 * NaN would poison the PSUM
            for j in range(4):
                nc.vector.memset(sgp[j][:], 1.0)
            # ones column for the softmax-denominator row of att@V
            for kb in range(NKB):
                nc.gpsimd.memset(v_sb[kb][:, :, D:D + 1], 1.0)

            def emit_proj_qk_oc(tt, oc, xt):
                ps = pspool.tile([P, QT], f32, tag="mm", bufs=2,
                                 name=f"pa{tt}_{oc}")
                for i in range(IC):
                    nc.tensor.matmul(
                        ps[:], wqk_sb[i][:, oc * P:(oc + 1) * P],
                        xt[i][:], start=(i == 0), stop=(i == IC - 1))
                # bias-add on the Scalar engine (out = in*1 + bias): keeps
                # the DVE queue short so the causal-mask multiplies that
                # gate att@V never sit behind a bias-add
                nc.scalar.activation(
                    qk_sb[oc][:, tt * QT:(tt + 1) * QT], ps[:], Identity,
                    bias=bqk_sb[:, oc:oc + 1], scale=1.0)

            def emit_proj_v_tb(tt, tb, xt):
                kb = tt * 4 + tb
                ps = pspool.tile([P, NH, D], f32, tag="mm", bufs=2,
                                 name=f"pb{tt}_{tb}")
                for i in range(IC):
                    nc.tensor.matmul(
                        ps[:, :, :], xt[i][:, tb * P:(tb + 1) * P],
                        wv_sb[i][:], start=(i == 0), stop=(i == IC - 1))
                nc.vector.tensor_copy(v_sb[kb][:, :, 0:D], ps[:, :, :])

            def proj_chunks(tt, xt):
                cs = [lambda oc=oc: emit_proj_qk_oc(tt, oc, xt)
                      for oc in range(8)]
                cs += [lambda tb=tb: emit_proj_v_tb(tt, tb, xt)
                       for tb in range(4)]
                return cs

            # deferred normalize emissions: [emit_fn, age]; emitted two
            # flush events after the pair completes so the DVE reciprocal
            # chain never stalls the PE queue.
            norm_q = []

            def service_norms(force=False):
                for item in list(norm_q):
                    item[1] += 1
                    if force or item[1] >= 2:
                        item[0]()
                        norm_q.remove(item)

            def emit_norm_pair(qtt, j, yrp_t):
                # broadcast the two reciprocal rows across 64 partitions each;
                # f32r bitcast keeps full PE rate without a bf16 staging pass
                btp = pspool.tile([P, QT], f32, tag="po", bufs=2,
                                  name=f"btp{qtt}_{j}")
                nc.tensor.matmul(btp[:], sel2[0:33, :], rgpm[j][0:33, :],
                                 start=True, stop=True)
                nc.vector.tensor_mul(
                    y_sb[j][:, qtt * QT:(qtt + 1) * QT], yrp_t[:], btp[:])

            def flush_attv(qtt, h, po_t, at, kbs, ns, c0s, os_, nkb, yrp_t):
                for kb, n, c0, o in zip(kbs, ns, c0s, os_):
                    nc.tensor.matmul(
                        po_t[:, c0:QT], v_sb[kb][:, h, :], at[:, o:o + n],
                        start=(kb == 0), stop=(kb == nkb - 1))
                if kbs[1] == nkb - 1:
                    # head complete: evict numerator + denominator, free PSUM.
                    # reciprocal+cast go before the numerator copy so the
                    # btp matmul's DVE chain is as short as possible.
                    j, r = h // 2, h % 2
                    nc.vector.tensor_copy(sgp[j][32 * r:32 * r + 1, :],
                                          po_t[D:D + 1, :])
                    if r == 1:
                        nc.vector.reciprocal_approx_fast(rgp[j][:], sgp[j][:])
                        nc.vector.tensor_copy(rgpm[j][:], rgp[j][:])
                    nc.vector.tensor_copy(yrp_t[r * D:(r + 1) * D, :],
                                          po_t[0:D, :])
                    if r == 1:
                        norm_q.append(
                            [lambda q=qtt, jj=j, y=yrp_t:
                             emit_norm_pair(q, jj, y), 0])

            # att@V runs two pairs behind the score/exp emission so the
            # exp -> mask chain has ~2 pairs of PE runway to complete in
            PEND_DEPTH = 2
            state = {"pend": []}

            def emit_head(qtt, h, yrp_map):
                nkb = (qtt + 1) * 4
                p0 = (h % 2) * D
                qt_i = h // 2
                kt_i = 4 + h // 2
                if h % 2 == 0:
                    yrp_map[(qtt, h // 2)] = yraw_pool.tile(
                        [P, QT], mdt, tag="yraw", name=f"yr{qtt}_{h // 2}")
                yrp_t = yrp_map[(qtt, h // 2)]
                po_t = pspool.tile([D + 1, QT], f32, tag="po", bufs=2,
                                   name=f"po{qtt}_{h}")
                for pi in range(nkb // 2):
                    kbs = (2 * pi, 2 * pi + 1)
                    ns, c0s = [], []
                    for kb in kbs:
                        e = kb * P - qtt * QT
                        c0s.append(max(e, 0))
                        ns.append(QT - max(e, 0))
                    # pack both live column ranges into one tile; each
                    # matmul's output must stay inside one 512-col bank
                    o0 = 0
                    o1 = ns[0] if ns[0] + ns[1] <= QT else QT
                    width = o1 + ns[1]
                    ps = pspool.tile([P, 2 * QT], f32, tag="ps", bufs=2,
                                     name=f"ps{qtt}_{h}_{pi}")
                    at = apool.tile([P, 2 * QT], mdt, tag="at",
                                    name=f"at{qtt}_{h}_{pi}")
                    for kb, n, c0, o in zip(kbs, ns, c0s, (o0, o1)):
                        nc.tensor.matmul(
                            ps[:, o:o + n],
                            qk_sb[kt_i][p0:p0 + D, kb * P:(kb + 1) * P],
                            qk_sb[qt_i][p0:p0 + D,
                                        qtt * QT + c0:(qtt + 1) * QT],
                            start=True, stop=True)
                    nc.scalar.activation(at[:, 0:width], ps[:, 0:width],
                                         Exp, scale=0.125)
                    for kb, n, c0, o in zip(kbs, ns, c0s, (o0, o1)):
                        if kb * P - qtt * QT >= 0:
                            # zero strict upper triangle; it never
                            # reaches past the first 128 live columns
                            m = min(n, P)
                            nc.vector.tensor_mul(at[:, o:o + m],
                                                 at[:, o:o + m],
                                                 maskf[:, 0:m])
                    if len(state["pend"]) >= PEND_DEPTH:
                        flush_attv(*state["pend"].pop(0))
                        service_norms()
                    state["pend"].append(
                        (qtt, h, po_t, at, kbs, ns, c0s, (o0, o1), nkb, yrp_t))

            def drain_pend():
                while state["pend"]:
                    flush_attv(*state["pend"].pop(0))
                    service_norms()

            def attention_phase(qtts, chunks=(), tail_after=None, prime=True):
                """Emit attention for one or two query tiles; two tiles are
                interleaved head-wise so each hides the other's exp/mask
                latency.  `chunks` are independent PE work: spread across all
                heads by default, or packed after slot `tail_after`."""
                yrp_map = {}
                if len(qtts) == 1:
                    slots = [(qtts[0], h) for h in range(NH)]
                else:
                    a, b = qtts
                    slots = [(a, 0), (a, 1), (b, 0), (a, 2), (b, 1), (a, 3),
                             (b, 2), (a, 4), (b, 3), (a, 5), (b, 4), (a, 6),
                             (b, 5), (a, 7), (b, 6), (b, 7)]
                n = len(slots)
                emitted = 0
                if tail_after is None and prime:
                    # prime the PE pipe with a couple of chunks so the first
                    # head's exp chain is covered
                    pre = min(2, len(chunks))
                    while emitted < pre:
                        chunks[emitted]()
                        emitted += 1
                for si, (qtt, h) in enumerate(slots):
                    emit_head(qtt, h, yrp_map)
                    if tail_after is None:
                        want = emitted if len(chunks) == 0 else max(
                            emitted, (si + 1) * len(chunks) // n)
                    elif si > tail_after:
                        want = ((si - tail_after) * len(chunks)
                                // (n - 1 - tail_after))
                    else:
                        want = 0
                    while emitted < want:
                        chunks[emitted]()
                        emitted += 1
                drain_pend()
                while emitted < len(chunks):
                    chunks[emitted]()
                    emitted += 1

            def emit_cproj_oc(tt, oc, ps=None, ics=(0, 1, 2, 3), evict=True):
                if ps is None:
                    ps = pspool.tile([P, QT], f32, tag="mm", bufs=2,
                                     name=f"pz{tt}_{oc}")
                for i in ics:
                    nc.tensor.matmul(
                        ps[:], wp_sb[i][:, oc * P:(oc + 1) * P],
                        y_sb[i][:, tt * QT:(tt + 1) * QT],
                        start=(i == 0), stop=(i == 3))
                if evict:
                    zt = zpool.tile([P, QT], mdt, tag="zt", name=f"zt{tt}_{oc}")
                    nc.vector.tensor_scalar_add(zt[:], ps[:],
                                                bpj_sb[:, oc:oc + 1])
                    q = nc.scalar if oc % 2 else nc.sync
                    q.dma_start(
                        zT[oc * P:(oc + 1) * P, tt * QT:(tt + 1) * QT], zt[:])
                return ps

            def emit_cproj(tt):
                # two chains' independent first-thirds run while the last
                # pair's normalize chain drains, then force the normalize.
                # oc2-7 (fully independent of the last pair until their own
                # ic3) go next so the queue never parks on the y mult.
                pss = {}
                for oc in (0, 1):
                    pss[oc] = emit_cproj_oc(tt, oc, ics=(0, 1, 2), evict=False)
                service_norms(force=True)
                for oc in range(8):
                    emit_cproj_oc(tt, oc, ps=pss.get(oc),
                                  ics=(3,) if oc in pss else (0, 1, 2, 3))

            # ---- fused schedule ----
            for oc in range(8):
                emit_proj_qk_oc(0, oc, xt0)
            for tb in range(4):
                emit_proj_v_tb(0, tb, xt0)
            # prefetch x(2)/x(3) now: their slot-reuse waits (on x(0)/x(1)
            # readers) resolve mid-attention(0), so the data is resident
            # well before the proj(2)/proj(3) chunks inside attention(1)
            xt2 = emit_x_dma(2)
            xt3 = emit_x_dma(3)
            # no priming for tile 0: x(1) is still in flight and a gated
            # chunk at the queue head would stall the ready first scores
            attention_phase([0], proj_chunks(1, xt1), prime=False)
            emit_cproj(0)
            attention_phase([1], proj_chunks(2, xt2) + proj_chunks(3, xt3))
            emit_cproj(1)
            cproj2 = [lambda oc=oc: emit_cproj_oc(2, oc) for oc in range(8)]
            attention_phase([2, 3], cproj2, tail_after=13)
            emit_cproj(3)
    nc.compile()
    return nc


def get_nc():
    if "nc" not in _nc_cache:
        _nc_cache["nc"] = _build_nc()
    return _nc_cache["nc"]


def _mm_np_dtype():
    if MM_DT == "bfloat16":
        import ml_dtypes
        return np.dtype(ml_dtypes.bfloat16)
    return np.dtype(np.float32)


def make_in_maps(x, Wqkv, bqkv, Wproj, bproj):
    x = np.asarray(x, np.float32)
    Wqkv = np.asarray(Wqkv, np.float32)
    bqkv = np.asarray(bqkv, np.float32)
    Wproj = np.asarray(Wproj, np.float32)
    bproj = np.asarray(bproj, np.float32)
    Wq, Wk, Wv = Wqkv[0:C], Wqkv[C:2 * C], Wqkv[2 * C:3 * C]
    bq, bk, bv = bqkv[0:C], bqkv[C:2 * C], bqkv[2 * C:3 * C]
    mdt = _mm_np_dtype()
    in_maps = []
    for b in range(B):
        xTb = np.ascontiguousarray(x[b].T.astype(mdt))
        for s in range(2):
            cols = slice(s * LC, (s + 1) * LC)
            wqkT = np.ascontiguousarray(
                np.concatenate([Wq[cols], Wk[cols]], 0).T.astype(mdt))
            bqk_ = np.concatenate([bq[cols], bk[cols]])
            wvT_ = np.ascontiguousarray(Wv[cols].T.astype(mdt))
            wpT_ = np.ascontiguousarray(Wproj[:, cols].T.astype(mdt))
            bp_eff = bv[cols] @ Wproj[:, cols].T
            if s == 0:
                bp_eff = bp_eff + bproj
            sel2_np = np.zeros((33, P), np.float32)
            sel2_np[0, 0:D] = 1.0
            sel2_np[32, D:P] = 1.0
            sel2_np = sel2_np.astype(mdt)
            in_maps.append({
                "xT": xTb,
                "wqkT": wqkT,
                "bqk": np.ascontiguousarray(bqk_.reshape(8, P).T),
                "wvT": wvT_,
                "wpT": wpT_,
                "bpj": np.ascontiguousarray(bp_eff.astype(np.float32).reshape(8, P).T),
                "sel2d": sel2_np,
            })
    return in_maps


def gather_out(results):
    out = np.empty((B, T, C), np.float32)
    for b in range(B):
        zt = (results[2 * b]["zT"].astype(np.float32)
              + results[2 * b + 1]["zT"].astype(np.float32))
        out[b] = zt.T
    return out


def kernel(x, Wqkv, bqkv, Wproj, bproj):
    from concourse.bass_utils import run_bass_kernel_spmd

    in_maps = make_in_maps(x, Wqkv, bqkv, Wproj, bproj)
    try:
        res = run_bass_kernel_spmd(get_nc(), in_maps, core_ids=list(range(8)))
    except Exception:
        # transient device faults have been observed once; retry a single time
        res = run_bass_kernel_spmd(get_nc(), in_maps, core_ids=list(range(8)))
    return gather_out(res.results)


# revision 41
# speedup vs baseline: 1.0356x; 1.0315x over previous
"""Causal self-attention (B=4, T=2048, C=1024, H=16) on 8 trn2 NeuronCores.

Sharding: core = (batch b, head-half s).  Each core computes q/k/v
projections for its 8 heads (weights pre-sliced/transposed on host),
causal flash-style attention with transposed score tiles, and a partial
(row-sharded) c_proj.  Host gather sums the two partials per batch.

v4: keeps the PE column stream dense through the latency-bound early
attention tiles.
 - proj(tt+1) is emitted as 12 chunks interleaved between attention(tt)
   heads: the small causal tiles (qtt=0/1) are exp/DVE-latency-bound, and
   the projection chunks give the PE independent work to chew on.
 - softmax reciprocal runs directly on the PSUM denominator row (one DVE
   op per head, no staging copies), and the 2-head broadcast matmul reads
   the f32 reciprocals via a float32r bitcast (full PE rate at 512 wide,
   no bf16 cast pass).
 - numerator eviction moved to the Scalar engine (activation Copy),
   freeing the DVE for masks + bias-adds.
 - input DMAs split/interleaved across the Sync+Scalar HWDGE queues;
   x(tt+1) is prefetched before attention(tt); odd c_proj output tiles
   drain on the Scalar queue.
"""

import os
import sys

sys.path.insert(0, "/opt/trn_rl_repo")

import numpy as np

B, T, C, H = 4, 2048, 1024, 16
D = 64          # head dim
NH = 8          # heads per core
LC = NH * D     # local channels = 512
P = 128
QT = 512        # query tile (also matmul moving free dim)
NQT = T // QT   # 4
NKB = T // P    # 16 key blocks
IC = C // P     # 8 input-channel blocks

# matmul input dtype: bfloat16 = full-rate PE mode,
# float32r = full-rate reduced-precision fp32, float32 = exact but 4x slower.
MM_DT = os.environ.get("BASS_ATTN_MM_DT", "bfloat16")

_nc_cache = {}


def _build_nc():
    from contextlib import ExitStack

    import concourse.bass as bass  # noqa: F401
    import concourse.mybir as mybir
    from concourse import bacc, tile

    f32 = mybir.dt.float32
    f32r = mybir.dt.float32r
    mdt = getattr(mybir.dt, MM_DT)
    Exp = mybir.ActivationFunctionType.Exp
    Identity = mybir.ActivationFunctionType.Identity
    is_ge = mybir.AluOpType.is_ge

    nc = bacc.Bacc("TRN2", target_bir_lowering=False, debug=False, num_devices=8)
    xT = nc.dram_tensor("xT", [C, T], mdt, kind="ExternalInput").ap()
    wqkT = nc.dram_tensor("wqkT", [C, 2 * LC], mdt, kind="ExternalInput").ap()
    bqk = nc.dram_tensor("bqk", [P, 2 * LC // P], f32, kind="ExternalInput").ap()
    wvT = nc.dram_tensor("wvT", [C, LC], mdt, kind="ExternalInput").ap()
    wpT = nc.dram_tensor("wpT", [LC, C], mdt, kind="ExternalInput").ap()
    bpj = nc.dram_tensor("bpj", [P, C // P], f32, kind="ExternalInput").ap()
    sel2d = nc.dram_tensor("sel2d", [33, P], mdt, kind="ExternalInput").ap()
    zT = nc.dram_tensor("zT", [C, T], mdt, kind="ExternalOutput").ap()

    with tile.TileContext(nc) as tc:
        with ExitStack() as st:
            persist = st.enter_context(tc.tile_pool(name="persist", bufs=1))
            # qk_sb: out-ch blocks 0-3 = q, 4-7 = k; [out-ch 128, tok 2048]
            qk_sb = [persist.tile([P, T], mdt, tag=f"qk{i}", name=f"qk{i}")
                     for i in range(8)]
            # v_sb[kb]: [tok 128, head 8, d 64 + ones col]
            v_sb = [persist.tile([P, NH, D + 1], mdt, tag=f"v{i}", name=f"v{i}")
                    for i in range(NKB)]
            # y_sb: attention out, [local-ch 128, tok 2048]; tile j = heads 2j,2j+1
            y_sb = [persist.tile([P, T], mdt, tag=f"y{i}", name=f"y{i}")
                    for i in range(4)]
            wqk_sb = [persist.tile([P, 2 * LC], mdt, tag=f"wqk{i}", name=f"wqk{i}")
                      for i in range(IC)]
            wv_sb = [persist.tile([P, LC], mdt, tag=f"wv{i}", name=f"wv{i}")
                     for i in range(IC)]
            wp_sb = [persist.tile([P, C], mdt, tag=f"wp{i}", name=f"wp{i}")
                     for i in range(4)]
            bqk_sb = persist.tile([P, 8], f32, tag="bqk", name="bqk")
            bpj_sb = persist.tile([P, 8], f32, tag="bpj", name="bpj")
            maskf = persist.tile([P, QT], mdt, tag="maskf", name="maskf")
            # head-pair select matrix for the denominator broadcast:
            # row 0 -> partitions 0-63, row 32 -> partitions 64-127 (rows
            # 1-31 are zero; 0/32 are the only legal DVE write bases)
            sel2 = persist.tile([33, P], mdt, tag="sel2", name="sel2")
            # per head pair j: softmax denominators staged at rows 0/32
            # (reciprocal must read SBUF: fed from PSUM it emits nothing)
            sgp = [persist.tile([33, QT], f32, tag=f"sg{i}", name=f"sg{i}")
                   for i in range(4)]
            rgp = [persist.tile([33, QT], f32, tag=f"rg{i}", name=f"rg{i}")
                   for i in range(4)]
            rgpm = [persist.tile([33, QT], mdt, tag=f"rgm{i}", name=f"rgm{i}")
                    for i in range(4)]

            yraw_pool = st.enter_context(tc.tile_pool(name="yraw", bufs=4))
            xpool = st.enter_context(tc.tile_pool(name="xs", bufs=2))
            apool = st.enter_context(tc.tile_pool(name="att", bufs=6))
            zpool = st.enter_context(tc.tile_pool(name="zev", bufs=3))
            # PSUM budget (8 banks): ps 2x2 + po/btp 2x1 + mm 2x1
            pspool = st.enter_context(tc.tile_pool(name="ps", bufs=2, space="PSUM"))

            # ---- input DMAs ----
            # sync + scalar are the two HWDGE queues; split the startup
            # traffic across both and interleave so proj(0)'s per-ic matmul
            # chain unlocks incrementally instead of after the whole 2MB.
            nc.sync.dma_start(bqk_sb[:], bqk)

            def emit_x_dma(tt):
                # sync queue: idle mid-kernel, so these issue immediately and
                # the 1MB transfer overlaps the running attention phase
                xt = [xpool.tile([P, QT], mdt, tag=f"x{i}", name=f"x{tt}_{i}")
                      for i in range(IC)]
                for i in range(IC):
                    nc.sync.dma_start(
                        xt[i][:], xT[i * P:(i + 1) * P, tt * QT:(tt + 1) * QT])
                return xt

            # descriptor-issue time (~0.6us each) dominates the startup
            # queues, so split by need-time: sync gets wqk-q + x(1), scalar
            # gets x(0) + wqk-k + wv + wp
            xt0 = [xpool.tile([P, QT], mdt, tag=f"x{i}", name=f"x0_{i}")
                   for i in range(IC)]
            for i in range(IC):
                nc.sync.dma_start(wqk_sb[i][:, 0:LC],
                                  wqkT[i * P:(i + 1) * P, 0:LC])
                nc.scalar.dma_start(
                    xt0[i][:], xT[i * P:(i + 1) * P, 0:QT])
            xt1 = emit_x_dma(1)
            for i in range(IC):
                nc.scalar.dma_start(wqk_sb[i][:, LC:2 * LC],
                                    wqkT[i * P:(i + 1) * P, LC:2 * LC])
            for i in range(IC):
                nc.scalar.dma_start(wv_sb[i][:], wvT[i * P:(i + 1) * P, :])
            for i in range(4):
                nc.scalar.dma_start(wp_sb[i][:], wpT[i * P:(i + 1) * P, :])
            nc.sync.dma_start(bpj_sb[:], bpj)
            nc.sync.dma_start(sel2[:], sel2d)

            # ---- one-time init ----
            # triangular mask (keep j >= p), shared by all diagonal blocks
            nc.vector.memset(maskf[:], 1.0)
            nc.gpsimd.affine_select(
                maskf[:], maskf[:], compare_op=is_ge, fill=0.0,
                base=0, pattern=[[1, QT]], channel_multiplier=-1)
            # keep the unwritten filler rows finite: the broadcast matmul
            # streams rows 0-32 and 0 * NaN would poison the PSUM
            for j in range(4):
                nc.vector.memset(sgp[j][:], 1.0)
            # ones column for the softmax-denominator row of att@V
            for kb in range(NKB):
                nc.gpsimd.memset(v_sb[kb][:, :, D:D + 1], 1.0)

            def emit_proj_qk_oc(tt, oc, xt, bias_dve=False):
                ps = pspool.tile([P, QT], f32, tag="mm", bufs=2,
                                 name=f"pa{tt}_{oc}")
                for i in range(IC):
                    nc.tensor.matmul(
                        ps[:], wqk_sb[i][:, oc * P:(oc + 1) * P],
                        xt[i][:], start=(i == 0), stop=(i == IC - 1))
                if bias_dve:
                    # startup path: the scalar queue is busy issuing DMA
                    # descriptors and would park the PSUM eviction
                    nc.vector.tensor_scalar_add(
                        qk_sb[oc][:, tt * QT:(tt + 1) * QT], ps[:],
                        bqk_sb[:, oc:oc + 1])
                else:
                    # bias-add on the Scalar engine (out = in*1 + bias):
                    # keeps the DVE queue short so the causal-mask multiplies
                    # that gate att@V never sit behind a bias-add
                    nc.scalar.activation(
                        qk_sb[oc][:, tt * QT:(tt + 1) * QT], ps[:], Identity,
                        bias=bqk_sb[:, oc:oc + 1], scale=1.0)

            def emit_proj_v_tb(tt, tb, xt):
                kb = tt * 4 + tb
                ps = pspool.tile([P, NH, D], f32, tag="mm", bufs=2,
                                 name=f"pb{tt}_{tb}")
                for i in range(IC):
                    nc.tensor.matmul(
                        ps[:, :, :], xt[i][:, tb * P:(tb + 1) * P],
                        wv_sb[i][:], start=(i == 0), stop=(i == IC - 1))
                nc.vector.tensor_copy(v_sb[kb][:, :, 0:D], ps[:, :, :])

            def proj_chunks(tt, xt):
                cs = [lambda oc=oc: emit_proj_qk_oc(tt, oc, xt)
                      for oc in range(8)]
                cs += [lambda tb=tb: emit_proj_v_tb(tt, tb, xt)
                       for tb in range(4)]
                return cs

            # deferred normalize emissions: [emit_fn, age]; emitted two
            # flush events after the pair completes so the DVE reciprocal
            # chain never stalls the PE queue.
            norm_q = []

            def service_norms(force=False):
                for item in list(norm_q):
                    item[1] += 1
                    if force or item[1] >= 2:
                        item[0]()
                        norm_q.remove(item)

            def emit_norm_pair(qtt, j, yrp_t):
                # broadcast the two reciprocal rows across 64 partitions each;
                # f32r bitcast keeps full PE rate without a bf16 staging pass
                btp = pspool.tile([P, QT], f32, tag="po", bufs=2,
                                  name=f"btp{qtt}_{j}")
                nc.tensor.matmul(btp[:], sel2[0:33, :], rgpm[j][0:33, :],
                                 start=True, stop=True)
                nc.vector.tensor_mul(
                    y_sb[j][:, qtt * QT:(qtt + 1) * QT], yrp_t[:], btp[:])

            def flush_attv(qtt, h, po_t, at, kbs, ns, c0s, os_, nkb, yrp_t):
                for kb, n, c0, o in zip(kbs, ns, c0s, os_):
                    nc.tensor.matmul(
                        po_t[:, c0:QT], v_sb[kb][:, h, :], at[:, o:o + n],
                        start=(kb == 0), stop=(kb == nkb - 1))
                if kbs[1] == nkb - 1:
                    # head complete: evict numerator + denominator, free PSUM.
                    # reciprocal+cast go before the numerator copy so the
                    # btp matmul's DVE chain is as short as possible.
                    j, r = h // 2, h % 2
                    nc.vector.tensor_copy(sgp[j][32 * r:32 * r + 1, :],
                                          po_t[D:D + 1, :])
                    if r == 1:
                        nc.vector.reciprocal_approx_fast(rgp[j][:], sgp[j][:])
                        nc.vector.tensor_copy(rgpm[j][:], rgp[j][:])
                    nc.vector.tensor_copy(yrp_t[r * D:(r + 1) * D, :],
                                          po_t[0:D, :])
                    if r == 1:
                        norm_q.append(
                            [lambda q=qtt, jj=j, y=yrp_t:
                             emit_norm_pair(q, jj, y), 0])

            # att@V runs two pairs behind the score/exp emission so the
            # exp -> mask chain has ~2 pairs of PE runway to complete in
            PEND_DEPTH = 2
            state = {"pend": []}

            def emit_head(qtt, h, yrp_map):
                nkb = (qtt + 1) * 4
                p0 = (h % 2) * D
                qt_i = h // 2
                kt_i = 4 + h // 2
                if h % 2 == 0:
                    yrp_map[(qtt, h // 2)] = yraw_pool.tile(
                        [P, QT], mdt, tag="yraw", name=f"yr{qtt}_{h // 2}")
                yrp_t = yrp_map[(qtt, h // 2)]
                po_t = pspool.tile([D + 1, QT], f32, tag="po", bufs=2,
                                   name=f"po{qtt}_{h}")
                for pi in range(nkb // 2):
                    kbs = (2 * pi, 2 * pi + 1)
                    ns, c0s = [], []
                    for kb in kbs:
                        e = kb * P - qtt * QT
                        c0s.append(max(e, 0))
                        ns.append(QT - max(e, 0))
                    # pack both live column ranges into one tile; each
                    # matmul's output must stay inside one 512-col bank
                    o0 = 0
                    o1 = ns[0] if ns[0] + ns[1] <= QT else QT
                    width = o1 + ns[1]
                    ps = pspool.tile([P, 2 * QT], f32, tag="ps", bufs=2,
                                     name=f"ps{qtt}_{h}_{pi}")
                    at = apool.tile([P, 2 * QT], mdt, tag="at",
                                    name=f"at{qtt}_{h}_{pi}")
                    for kb, n, c0, o in zip(kbs, ns, c0s, (o0, o1)):
                        nc.tensor.matmul(
                            ps[:, o:o + n],
                            qk_sb[kt_i][p0:p0 + D, kb * P:(kb + 1) * P],
                            qk_sb[qt_i][p0:p0 + D,
                                        qtt * QT + c0:(qtt + 1) * QT],
                            start=True, stop=True)
                    nc.scalar.activation(at[:, 0:width], ps[:, 0:width],
                                         Exp, scale=0.125)
                    for kb, n, c0, o in zip(kbs, ns, c0s, (o0, o1)):
                        if kb * P - qtt * QT >= 0:
                            # zero strict upper triangle; it never
                            # reaches past the first 128 live columns
                            m = min(n, P)
                            nc.vector.tensor_mul(at[:, o:o + m],
                                                 at[:, o:o + m],
                                                 maskf[:, 0:m])
                    if len(state["pend"]) >= PEND_DEPTH:
                        flush_attv(*state["pend"].pop(0))
                        service_norms()
                    state["pend"].append(
                        (qtt, h, po_t, at, kbs, ns, c0s, (o0, o1), nkb, yrp_t))

            def drain_pend():
                while state["pend"]:
                    flush_attv(*state["pend"].pop(0))
                    service_norms()

            def attention_phase(qtts, chunks=(), tail_after=None, prime=True):
                """Emit attention for one or two query tiles; two tiles are
                interleaved head-wise so each hides the other's exp/mask
                latency.  `chunks` are independent PE work: spread across all
                heads by default, or packed after slot `tail_after`."""
                yrp_map = {}
                if len(qtts) == 1:
                    slots = [(qtts[0], h) for h in range(NH)]
                else:
                    a, b = qtts
                    slots = [(a, 0), (a, 1), (b, 0), (a, 2), (b, 1), (a, 3),
                             (b, 2), (a, 4), (b, 3), (a, 5), (b, 4), (a, 6),
                             (b, 5), (a, 7), (b, 6), (b, 7)]
                n = len(slots)
                emitted = 0
                if tail_after is None and prime:
                    # prime the PE pipe with a couple of chunks so the first
                    # head's exp chain is covered
                    pre = min(2, len(chunks))
                    while emitted < pre:
                        chunks[emitted]()
                        emitted += 1
                for si, (qtt, h) in enumerate(slots):
                    emit_head(qtt, h, yrp_map)
                    if tail_after is None:
                        want = emitted if len(chunks) == 0 else max(
                            emitted, (si + 1) * len(chunks) // n)
                    elif si > tail_after:
                        want = ((si - tail_after) * len(chunks)
                                // (n - 1 - tail_after))
                    else:
                        want = 0
                    while emitted < want:
                        chunks[emitted]()
                        emitted += 1
                drain_pend()
                while emitted < len(chunks):
                    chunks[emitted]()
                    emitted += 1

            def emit_cproj_oc(tt, oc, ps=None, ics=(0, 1, 2, 3), evict=True):
                if ps is None:
                    ps = pspool.tile([P, QT], f32, tag="mm", bufs=2,
                                     name=f"pz{tt}_{oc}")
                for i in ics:
                    nc.tensor.matmul(
                        ps[:], wp_sb[i][:, oc * P:(oc + 1) * P],
                        y_sb[i][:, tt * QT:(tt + 1) * QT],
                        start=(i == 0), stop=(i == 3))
                if evict:
                    zt = zpool.tile([P, QT], mdt, tag="zt", name=f"zt{tt}_{oc}")
                    nc.vector.tensor_scalar_add(zt[:], ps[:],
                                                bpj_sb[:, oc:oc + 1])
                    q = nc.scalar if oc % 2 else nc.sync
                    q.dma_start(
                        zT[oc * P:(oc + 1) * P, tt * QT:(tt + 1) * QT], zt[:])
                return ps

            def emit_cproj(tt):
                # two chains' independent first-thirds run while the last
                # pair's normalize chain drains, then force the normalize.
                # oc2-7 (fully independent of the last pair until their own
                # ic3) go next so the queue never parks on the y mult.
                pss = {}
                for oc in (0, 1):
                    pss[oc] = emit_cproj_oc(tt, oc, ics=(0, 1, 2), evict=False)
                service_norms(force=True)
                for oc in range(8):
                    emit_cproj_oc(tt, oc, ps=pss.get(oc),
                                  ics=(3,) if oc in pss else (0, 1, 2, 3))

            # ---- fused schedule ----
            for oc in range(8):
                emit_proj_qk_oc(0, oc, xt0, bias_dve=True)
            for tb in range(4):
                emit_proj_v_tb(0, tb, xt0)
            # prefetch x(2)/x(3) now: their slot-reuse waits (on x(0)/x(1)
            # readers) resolve mid-attention(0), so the data is resident
            # well before the proj(2)/proj(3) chunks inside attention(1)
            xt2 = emit_x_dma(2)
            xt3 = emit_x_dma(3)
            # no priming for tile 0: x(1) is still in flight and a gated
            # chunk at the queue head would stall the ready first scores
            attention_phase([0], proj_chunks(1, xt1), prime=False)
            emit_cproj(0)
            attention_phase([1], proj_chunks(2, xt2) + proj_chunks(3, xt3))
            emit_cproj(1)
            cproj2 = [lambda oc=oc: emit_cproj_oc(2, oc) for oc in range(8)]
            attention_phase([2, 3], cproj2, tail_after=13)
            emit_cproj(3)
    nc.compile()
    return nc


def get_nc():
    if "nc" not in _nc_cache:
        _nc_cache["nc"] = _build_nc()
    return _nc_cache["nc"]


def _mm_np_dtype():
    if MM_DT == "bfloat16":
        import ml_dtypes
        return np.dtype(ml_dtypes.bfloat16)
    return np.dtype(np.float32)


def make_in_maps(x, Wqkv, bqkv, Wproj, bproj):
    x = np.asarray(x, np.float32)
    Wqkv = np.asarray(Wqkv, np.float32)
    bqkv = np.asarray(bqkv, np.float32)
    Wproj = np.asarray(Wproj, np.float32)
    bproj = np.asarray(bproj, np.float32)
    Wq, Wk, Wv = Wqkv[0:C], Wqkv[C:2 * C], Wqkv[2 * C:3 * C]
    bq, bk, bv = bqkv[0:C], bqkv[C:2 * C], bqkv[2 * C:3 * C]
    mdt = _mm_np_dtype()
    in_maps = []
    for b in range(B):
        xTb = np.ascontiguousarray(x[b].T.astype(mdt))
        for s in range(2):
            cols = slice(s * LC, (s + 1) * LC)
            wqkT = np.ascontiguousarray(
                np.concatenate([Wq[cols], Wk[cols]], 0).T.astype(mdt))
            bqk_ = np.concatenate([bq[cols], bk[cols]])
            wvT_ = np.ascontiguousarray(Wv[cols].T.astype(mdt))
            wpT_ = np.ascontiguousarray(Wproj[:, cols].T.astype(mdt))
            bp_eff = bv[cols] @ Wproj[:, cols].T
            if s == 0:
                bp_eff = bp_eff + bproj
            sel2_np = np.zeros((33, P), np.float32)
            sel2_np[0, 0:D] = 1.0
            sel2_np[32, D:P] = 1.0
            sel2_np = sel2_np.astype(mdt)
            in_maps.append({
                "xT": xTb,
                "wqkT": wqkT,
                "bqk": np.ascontiguousarray(bqk_.reshape(8, P).T),
                "wvT": wvT_,
                "wpT": wpT_,
                "bpj": np.ascontiguousarray(bp_eff.astype(np.float32).reshape(8, P).T),
                "sel2d": sel2_np,
            })
    return in_maps


def gather_out(results):
    out = np.empty((B, T, C), np.float32)
    for b in range(B):
        zt = (results[2 * b]["zT"].astype(np.float32)
              + results[2 * b + 1]["zT"].astype(np.float32))
        out[b] = zt.T
    return out


def kernel(x, Wqkv, bqkv, Wproj, bproj):
    from concourse.bass_utils import run_bass_kernel_spmd

    in_maps = make_in_maps(x, Wqkv, bqkv, Wproj, bproj)
    try:
        res = run_bass_kernel_spmd(get_nc(), in_maps, core_ids=list(range(8)))
    except Exception:
        # transient device faults have been observed once; retry a single time
        res = run_bass_kernel_spmd(get_nc(), in_maps, core_ids=list(range(8)))
    return gather_out(res.results)
